# revision 9
# baseline (speedup 1.0000x reference)
"""Self-contained Trainium2 Bass kernel for the AttentionBlock problem.

Shapes (hardcoded): x [8, 256, 64, 64] fp32, Wq/Wk [32, 256], bq/bk [32],
Wv [256, 256], bv [256], gamma [1].

Sharding: data-parallel over batch - each of the 8 NeuronCores computes the
full 4096x4096 attention for one batch element. No collectives.

Per-core algorithm (C=256, C8=32, N=4096), fully SBUF-resident.
Pipeline unit = "group": 2 key tiles x 512-query window (N=512 moving
operands keep every matmul streaming-bound; shorter moving lengths are
LDWEIGHTS-bound at ~131 ns/matmul).
  QK   2 row-packed K=32 bf16 matmuls (tile_position 0/32) -> one psum
       tile [128, 2, 512] (2 banks, double buffered)
  exp  one FD=1024 ACT instruction psum -> pt bf16
  acc  acc += pt on DVE (bf16 2x) - per-partition rowsum partials
  AV   4 bf16 matmuls (2 jt x 2 ch) accumulate v.T@p into av [128,2,512]
Per 512-query window (16 groups): rowsum = ones.T @ acc (2 matmuls into a
psum tile STOLEN from the QK pool's rotation - the buffer was just drained
by exp, so no extra banks), rinv = recip(rowsum), out = av*rinv +
(gamma*bv + x) with the epilogue reading av straight out of PSUM.  gamma
is folded into Wv.  The q/k projections run as one fused chain (wq|wk in
one 64-wide stationary), and the x2 replication needed for the row-packed
QK is done by SBUF-to-SBUF DMAs on otherwise-idle queues.  All projection
/ rowsum psum scratch is allocated from the QK pool's tag rotation, so
PSUM is exactly 8 banks: QK 2x2 + AV accumulators 2x2.
"""

import sys

import numpy as np

if "/opt/trn_rl_repo" not in sys.path:
    sys.path.insert(0, "/opt/trn_rl_repo")

import concourse.bass as bass
import concourse.bacc as bacc
import concourse.tile as tile
from concourse import mybir
from concourse.bass_utils import run_bass_kernel_spmd
from concourse.masks import make_identity

F32 = mybir.dt.float32
BF16 = mybir.dt.bfloat16

C = 256
C8 = 32
P = 128
CH = C // P  # 2 channel chunks
IW = 512     # query-window size


def build_attention_nc(n: int = 4096) -> bass.Bass:
    """Build the single-core Bass program (SPMD across 8 cores)."""
    assert n % IW == 0
    NW = n // IW        # query windows (8)
    JT = n // P         # key tiles (32)
    GPW = JT // 2       # groups per window (16)
    NG = NW * GPW       # total groups (128)
    NH = n // 2         # half of the token dim (x loaded as 2 halves)

    nc = bacc.Bacc("TRN2", target_bir_lowering=False)
    x_d = nc.declare_dram_parameter("x", [C, n], F32, isOutput=False)
    wq_d = nc.declare_dram_parameter("Wq", [C8, C], F32, isOutput=False)
    bq_d = nc.declare_dram_parameter("bq", [C8], F32, isOutput=False)
    wk_d = nc.declare_dram_parameter("Wk", [C8, C], F32, isOutput=False)
    bk_d = nc.declare_dram_parameter("bk", [C8], F32, isOutput=False)
    wv_d = nc.declare_dram_parameter("Wv", [C, C], F32, isOutput=False)
    bv_d = nc.declare_dram_parameter("bv", [C], F32, isOutput=False)
    gamma_d = nc.declare_dram_parameter("gamma", [1], F32, isOutput=False)
    out_d = nc.declare_dram_parameter("out", [C, n], F32, isOutput=True)

    with tile.TileContext(nc) as tc:
        with (
            tc.tile_pool(name="const", bufs=1) as const,
            tc.tile_pool(name="xpool", bufs=1) as xpool,
            tc.tile_pool(name="qkpool", bufs=1) as qkpool,
            tc.tile_pool(name="vtpool", bufs=1) as vtpool,
            tc.tile_pool(name="ptpool", bufs=3) as ptpool,
            tc.tile_pool(name="accpool", bufs=2) as accpool,
            tc.tile_pool(name="smallwork", bufs=4) as smallwork,
            tc.tile_pool(name="outpool", bufs=6) as outpool,
            tc.tile_pool(name="pe_ps", bufs=2, space="PSUM") as pe_ps,  # 2x2 banks
            tc.tile_pool(name="av_ps", bufs=2, space="PSUM") as av_ps,  # 2x2 banks
        ):
            # ---------------- setup: loads ----------------
            # warm the ACT exp table immediately
            warm_in = const.tile([P, 1], F32, tag="warmin")
            nc.gpsimd.memset(warm_in, 0.0)
            warm_out = const.tile([P, 1], F32, tag="warmout")
            nc.scalar.activation(warm_out, warm_in, mybir.ActivationFunctionType.Exp)

            ident = const.tile([P, P], F32, tag="ident")
            make_identity(nc, ident)

            ones_bf = const.tile([P, P], BF16, tag="ones")
            nc.vector.memset(ones_bf, 1.0)

            # x loads: two big [128, NH] DMAs per channel, ch0 on the sync
            # queue, ch1 on gpsimd (x_lo ch1 first - it gates the first
            # xcast).  The gamma/bias loads ride the idle scalar queue.
            x_lo = xpool.tile([P, CH, NH], F32, tag="xlo")
            x_hi = xpool.tile([P, CH, NH], F32, tag="xhi")
            nc.sync.dma_start(out=x_lo[:, 0, :], in_=x_d[0:P, 0:NH])
            nc.gpsimd.dma_start(out=x_lo[:, 1, :], in_=x_d[P : 2 * P, 0:NH])
            wq_stage = const.tile([C8, C], F32, tag="wqs")
            nc.gpsimd.dma_start(out=wq_stage, in_=wq_d[:, :])
            wk_stage = const.tile([C8, C], F32, tag="wks")
            nc.gpsimd.dma_start(out=wk_stage, in_=wk_d[:, :])
            wv_stage = const.tile([P, CH, C], F32, tag="wvs")
            nc.gpsimd.dma_start(
                out=wv_stage, in_=wv_d[:, :].rearrange("(a p) c -> p a c", p=P)
            )
            nc.sync.dma_start(out=x_hi[:, 0, :], in_=x_d[0:P, NH:n])
            nc.gpsimd.dma_start(out=x_hi[:, 1, :], in_=x_d[P : 2 * P, NH:n])
            bq_sb = const.tile([C8, 1], F32, tag="bq")
            nc.scalar.dma_start(
                out=bq_sb, in_=bq_d[:].rearrange("(p one) -> p one", one=1)
            )
            bk_sb = const.tile([C8, 1], F32, tag="bk")
            nc.scalar.dma_start(
                out=bk_sb, in_=bk_d[:].rearrange("(p one) -> p one", one=1)
            )
            bv_sb = const.tile([P, CH], F32, tag="bv")
            nc.scalar.dma_start(
                out=bv_sb, in_=bv_d[:].rearrange("(ch p) -> p ch", p=P)
            )
            gamma_ap = gamma_d[:]
            gamma_sb = const.tile([P, 1], F32, tag="gamma")
            nc.scalar.dma_start(
                out=gamma_sb,
                in_=bass.AP(
                    tensor=gamma_ap.tensor, offset=gamma_ap.offset,
                    ap=[[0, P], gamma_ap.ap[0]],
                ),
            )
            gbv = const.tile([P, CH], F32, tag="gbv")
            xb_lo = xpool.tile([P, CH, NH], BF16, tag="xblo")
            xb_hi = xpool.tile([P, CH, NH], BF16, tag="xbhi")

            def x_win(iw):  # fp32 residual slice [P, CH, IW]
                t, off = (x_lo, iw * IW) if iw * IW < NH else (x_hi, iw * IW - NH)
                return t[:, :, off : off + IW]

            def xb_win(iw):  # bf16 slice [P, CH, IW]
                t, off = (xb_lo, iw * IW) if iw * IW < NH else (xb_hi, iw * IW - NH)
                return t[:, :, off : off + IW]

            def emit_xcast(iw):
                nc.vector.tensor_copy(xb_win(iw), x_win(iw))

            # ------------- weight transposes (bf16) -------------
            # wqkt[c, ch, 0:32] = wq^T chunk, wqkt[c, ch, 32:64] = wk^T chunk,
            # so one matmul chain projects q and k together.
            wqkt = const.tile([P, CH, 2 * C8], BF16, tag="wqkt")
            for ch in range(CH):
                ps_tqk = pe_ps.tile([P, 2 * C8], F32, tag="peps", name=f"ps_tqk{ch}")
                nc.tensor.transpose(
                    ps_tqk[:, 0:C8], wq_stage[:, bass.ts(ch, P)], ident[:C8, :C8]
                )
                nc.tensor.transpose(
                    ps_tqk[:, C8 : 2 * C8], wk_stage[:, bass.ts(ch, P)],
                    ident[:C8, :C8]
                )
                nc.vector.tensor_copy(wqkt[:, ch, :], ps_tqk)

            emit_xcast(0)

            # wvt[c, ci, o] = gamma * Wv[o, ci*128+c], bf16
            wvt = const.tile([P, CH, C], BF16, tag="wvt")
            for ci in range(CH):
                for oi in range(CH):
                    pool, ptag = (pe_ps, "peps") if oi == 0 else (av_ps, "avps")
                    ps_tv = pool.tile([P, P], F32, tag=ptag, name=f"ps_tv{ci}{oi}")
                    nc.tensor.transpose(
                        ps_tv, wv_stage[:, oi, bass.ts(ci, P)], ident
                    )
                    nc.vector.tensor_scalar_mul(
                        wvt[:, ci, bass.ts(oi, P)], ps_tv, gamma_sb
                    )

            # ---------------- projections ----------------
            # q4/k4: [64, n] bf16, q/k replicated x2 across partition groups
            # for the 2-way row-packed QK matmuls.  One fused chain projects
            # q and k together into qk_s; idle DMA queues do the replication.
            qk_s = qkpool.tile([2 * C8, n], BF16, tag="qks")
            q4 = qkpool.tile([2 * C8, n], BF16, tag="q4")
            k4 = qkpool.tile([2 * C8, n], BF16, tag="k4")
            bqk_sb = const.tile([2 * C8, 1], F32, tag="bqk")

            def emit_qkproj(iw):
                win = bass.ts(iw, IW)
                xbw = xb_win(iw)
                ps_qk = pe_ps.tile([P, IW], F32, tag="peps", name=f"ps_qk_{iw}")
                for ch in range(CH):
                    nc.tensor.matmul(
                        ps_qk[0 : 2 * C8, :], wqkt[:, ch, :], xbw[:, ch, :],
                        start=(ch == 0), stop=(ch == CH - 1),
                    )
                nc.vector.tensor_scalar_add(
                    qk_s[:, win], ps_qk[0 : 2 * C8, :], bqk_sb
                )
                # replicate into the packed layouts on idle DMA queues
                nc.sync.dma_start(out=q4[0:C8, win], in_=qk_s[0:C8, win])
                nc.gpsimd.dma_start(out=q4[C8 : 2 * C8, win], in_=qk_s[0:C8, win])
                nc.sync.dma_start(out=k4[0:C8, win], in_=qk_s[C8 : 2 * C8, win])
                nc.gpsimd.dma_start(
                    out=k4[C8 : 2 * C8, win], in_=qk_s[C8 : 2 * C8, win]
                )

            # vT per key tile: vt[jt][p, c] = gamma * (Wv x)[c, jt*128+p], bf16
            vt = [None] * JT

            def emit_vproj(jt, cast_on_act=False):
                vtt = vtpool.tile([P, C], BF16, tag=f"vt{jt}", name=f"vt{jt}")
                ps_v = pe_ps.tile([P, C], F32, tag="peps", name=f"ps_v{jt}")
                iww, off = (jt * P) // IW, (jt * P) % IW
                xbw = xb_win(iww)
                for ch in range(CH):
                    nc.tensor.matmul(
                        ps_v,
                        xbw[:, ch, off : off + P],
                        wvt[:, ch, :],
                        start=(ch == 0), stop=(ch == CH - 1),
                    )
                if cast_on_act:
                    nc.scalar.copy(vtt, ps_v)
                else:
                    nc.vector.tensor_copy(vtt, ps_v)
                vt[jt] = vtt

            nc.vector.tensor_copy(bqk_sb[0:C8, :], bq_sb)
            nc.vector.tensor_copy(bqk_sb[C8 : 2 * C8, :], bk_sb)
            emit_qkproj(0)
            for jt in range(4):
                emit_vproj(jt)
            emit_xcast(1)
            emit_qkproj(1)
            nc.vector.tensor_scalar_mul(gbv, bv_sb, gamma_sb)
            for jt in range(4, 8):
                emit_vproj(jt)
            vjt_late = list(range(8, JT))
            qk_late = list(range(2, NW))
            xc_late = list(range(2, NW))

            # ---------------- main pipeline ----------------
            state = {}
            pse_hist = {}

            def emit_group(g):
                iw, gg = divmod(g, GPW)
                win = bass.ts(iw, IW)
                if gg == 0:
                    state[iw] = {
                        "av": av_ps.tile(
                            [P, CH, IW], F32, tag="avps", name=f"av_{iw}"
                        ),
                        "acc": accpool.tile(
                            [P, 2, IW], BF16, tag="acc", name=f"acc_{iw}"
                        ),
                    }
                ps_e = pe_ps.tile([P, 2, IW], F32, tag="peps", name=f"ps_e{g}")
                pse_hist[g] = ps_e
                pse_hist.pop(g - 4, None)
                for m in range(2):
                    jt = 2 * gg + m
                    nc.tensor.matmul(
                        ps_e[:, m, :],
                        k4[m * C8 : (m + 1) * C8, bass.ts(jt, P)],
                        q4[m * C8 : (m + 1) * C8, win],
                        start=True, stop=True,
                        tile_position=(m * C8, 0),
                    )
                pt = ptpool.tile([P, 2, IW], BF16, tag="pt", name=f"pt{g}")
                nc.scalar.activation(pt, ps_e, mybir.ActivationFunctionType.Exp)
                acc = state[iw]["acc"]
                if gg == 0:
                    nc.vector.tensor_copy(acc, pt)
                else:
                    nc.vector.tensor_add(acc, acc, pt)
                return pt

            def emit_av(g, pt):
                iw, gg = divmod(g, GPW)
                av = state[iw]["av"]
                for m in range(2):
                    jt = 2 * gg + m
                    for ch in range(CH):
                        nc.tensor.matmul(
                            av[:, ch, :],
                            vt[jt][:, bass.ts(ch, P)],
                            pt[:, m, :],
                            start=(gg == 0 and m == 0),
                            stop=(gg == GPW - 1 and m == 1),
                            skip_group_check=True,
                        )

            def emit_epilogue(iw, cur_g):
                st = state.pop(iw)
                acc, av = st["acc"], st["av"]
                win = bass.ts(iw, IW)
                # rowsum goes into the previous group's ps_e tile: its exp
                # has already drained it, and the tile's next QK writer is
                # two groups out, past the reciprocal read below.  Reusing a
                # live tile (instead of allocating) keeps the pe_ps rotation
                # parity intact, so no group ever single-buffers.
                host = max(k for k in pse_hist if k <= cur_g - 1)
                ps_r = pse_hist[host][:, 0, :]
                for s in range(2):
                    nc.tensor.matmul(
                        ps_r, ones_bf, acc[:, s, :],
                        start=(s == 0), stop=(s == 1),
                    )
                rinv = smallwork.tile([P, IW], F32, tag="rinv", name=f"rinv{iw}")
                nc.vector.reciprocal_approx_fast(rinv, ps_r)
                xw = x_win(iw)
                for ch in range(CH):
                    o_sb = outpool.tile([P, IW], F32, tag="osb", name=f"osb{ch}_{iw}")
                    nc.vector.tensor_mul(o_sb, av[:, ch, :], rinv)
                    nc.vector.scalar_tensor_tensor(
                        out=o_sb, in0=o_sb, scalar=gbv[:, ch : ch + 1],
                        in1=xw[:, ch, :],
                        op0=mybir.AluOpType.add, op1=mybir.AluOpType.add,
                    )
                    eng = nc.sync if ch == 0 else nc.gpsimd
                    eng.dma_start(
                        out=out_d[ch * P : (ch + 1) * P, win], in_=o_sb
                    )

            pts = [None] * NG
            for g in range(NG + 1):
                if g < NG:
                    for _ in range(2):
                        if xc_late:
                            emit_xcast(xc_late.pop(0))
                    pts[g] = emit_group(g)
                if g > 0:
                    emit_av(g - 1, pts[g - 1])
                    pts[g - 1] = None
                if g < NG:
                    # drips go AFTER emit_av so their psum-rotation steals
                    # never stall the PE ahead of the AV matmuls
                    for _ in range(2):
                        if qk_late:
                            emit_qkproj(qk_late.pop(0))
                    for dv in range(3):
                        if vjt_late:
                            emit_vproj(vjt_late.pop(0), cast_on_act=(dv == 0))
                if g > 0 and g - 1 >= GPW + 1 and (g - 1) % GPW == 1:
                    emit_epilogue((g - 1) // GPW - 1, g)
            emit_epilogue(NW - 1, NG)

    nc.finalize()
    return nc


_NC_CACHE: dict[int, bass.Bass] = {}


def _get_nc(n: int) -> bass.Bass:
    if n not in _NC_CACHE:
        _NC_CACHE[n] = build_attention_nc(n)
    return _NC_CACHE[n]


def kernel(x, Wq, bq, Wk, bk, Wv, bv, gamma):
    B, c, h, w = x.shape
    n = h * w
    assert B == 8 and c == C
    nc = _get_nc(n)
    xf = np.ascontiguousarray(np.asarray(x, dtype=np.float32).reshape(B, c, n))
    common = {
        "Wq": np.ascontiguousarray(np.asarray(Wq, dtype=np.float32)),
        "bq": np.ascontiguousarray(np.asarray(bq, dtype=np.float32)),
        "Wk": np.ascontiguousarray(np.asarray(Wk, dtype=np.float32)),
        "bk": np.ascontiguousarray(np.asarray(bk, dtype=np.float32)),
        "Wv": np.ascontiguousarray(np.asarray(Wv, dtype=np.float32)),
        "bv": np.ascontiguousarray(np.asarray(bv, dtype=np.float32)),
        "gamma": np.ascontiguousarray(np.asarray(gamma, dtype=np.float32)),
    }
    in_maps = [{"x": xf[b], **common} for b in range(B)]
    res = run_bass_kernel_spmd(nc, in_maps, core_ids=list(range(B)))
    out = np.stack([res.results[b]["out"].reshape(c, h, w) for b in range(B)])
    return out.astype(np.float32)


# revision 10
# speedup vs baseline: 1.0282x; 1.0282x over previous
"""Self-contained Trainium2 Bass kernel for the AttentionBlock problem.

Shapes (hardcoded): x [8, 256, 64, 64] fp32, Wq/Wk [32, 256], bq/bk [32],
Wv [256, 256], bv [256], gamma [1].

Sharding: data-parallel over batch - each of the 8 NeuronCores computes the
full 4096x4096 attention for one batch element. No collectives.

Per-core algorithm (C=256, C8=32, N=4096), fully SBUF-resident.
Pipeline unit = "group": 2 key tiles x 512-query window (N=512 moving
operands keep every matmul streaming-bound; shorter moving lengths are
LDWEIGHTS-bound at ~131 ns/matmul).
  QK   2 row-packed K=32 bf16 matmuls (tile_position 0/32) -> one psum
       tile [128, 2, 512] (2 banks, double buffered)
  exp  one FD=1024 ACT instruction psum -> pt bf16
  acc  acc += pt on DVE (bf16 2x) - per-partition rowsum partials
  AV   4 bf16 matmuls (2 jt x 2 ch) accumulate v.T@p into av [128,2,512]
Per 512-query window (16 groups): rowsum = ones.T @ acc (2 matmuls into a
psum tile STOLEN from the QK pool's rotation - the buffer was just drained
by exp, so no extra banks), rinv = recip(rowsum), out = av*rinv +
(gamma*bv + x) with the epilogue reading av straight out of PSUM.  gamma
is folded into Wv.  The q/k projections run as one fused chain (wq|wk in
one 64-wide stationary), and the x2 replication needed for the row-packed
QK is done by SBUF-to-SBUF DMAs on otherwise-idle queues.  All projection
/ rowsum psum scratch is allocated from the QK pool's tag rotation, so
PSUM is exactly 8 banks: QK 2x2 + AV accumulators 2x2.
"""

import sys

import numpy as np

if "/opt/trn_rl_repo" not in sys.path:
    sys.path.insert(0, "/opt/trn_rl_repo")

import concourse.bass as bass
import concourse.bacc as bacc
import concourse.tile as tile
from concourse import mybir
from concourse.bass_utils import run_bass_kernel_spmd
from concourse.masks import make_identity

F32 = mybir.dt.float32
BF16 = mybir.dt.bfloat16

C = 256
C8 = 32
P = 128
CH = C // P  # 2 channel chunks
IW = 512     # query-window size


def build_attention_nc(n: int = 4096) -> bass.Bass:
    """Build the single-core Bass program (SPMD across 8 cores)."""
    assert n % IW == 0
    NW = n // IW        # query windows (8)
    JT = n // P         # key tiles (32)
    GPW = JT // 2       # groups per window (16)
    NG = NW * GPW       # total groups (128)
    NH = n // 2         # half of the token dim (x loaded as 2 halves)

    nc = bacc.Bacc("TRN2", target_bir_lowering=False)
    x_d = nc.declare_dram_parameter("x", [C, n], F32, isOutput=False)
    wq_d = nc.declare_dram_parameter("Wq", [C8, C], F32, isOutput=False)
    bq_d = nc.declare_dram_parameter("bq", [C8], F32, isOutput=False)
    wk_d = nc.declare_dram_parameter("Wk", [C8, C], F32, isOutput=False)
    bk_d = nc.declare_dram_parameter("bk", [C8], F32, isOutput=False)
    wv_d = nc.declare_dram_parameter("Wv", [C, C], F32, isOutput=False)
    bv_d = nc.declare_dram_parameter("bv", [C], F32, isOutput=False)
    gamma_d = nc.declare_dram_parameter("gamma", [1], F32, isOutput=False)
    out_d = nc.declare_dram_parameter("out", [C, n], F32, isOutput=True)

    with tile.TileContext(nc) as tc:
        with (
            tc.tile_pool(name="const", bufs=1) as const,
            tc.tile_pool(name="xpool", bufs=1) as xpool,
            tc.tile_pool(name="qkpool", bufs=1) as qkpool,
            tc.tile_pool(name="vtpool", bufs=1) as vtpool,
            tc.tile_pool(name="ptpool", bufs=3) as ptpool,
            tc.tile_pool(name="accpool", bufs=2) as accpool,
            tc.tile_pool(name="smallwork", bufs=4) as smallwork,
            tc.tile_pool(name="outpool", bufs=6) as outpool,
            tc.tile_pool(name="pe_ps", bufs=2, space="PSUM") as pe_ps,  # 2x2 banks
            tc.tile_pool(name="av_ps", bufs=2, space="PSUM") as av_ps,  # 2x2 banks
        ):
            # ---------------- setup: loads ----------------
            # warm the ACT exp table immediately
            warm_in = const.tile([P, 1], F32, tag="warmin")
            nc.gpsimd.memset(warm_in, 0.0)
            warm_out = const.tile([P, 1], F32, tag="warmout")
            nc.scalar.activation(warm_out, warm_in, mybir.ActivationFunctionType.Exp)

            ident = const.tile([P, P], F32, tag="ident")
            make_identity(nc, ident)

            ones_bf = const.tile([P, P], BF16, tag="ones")
            nc.vector.memset(ones_bf, 1.0)

            # tiny bias loads go FIRST so their packets clear the DMA rings
            # before the x flood; bq/bk are [32,1] (32 packets), bv is loaded
            # [2,128] (2 packets) and transposed on-chip later.  The gamma
            # partition-broadcast (128 tiny packets) is slow but nothing on
            # the critical path consumes it.
            bq_sb = const.tile([C8, 1], F32, tag="bq")
            nc.scalar.dma_start(
                out=bq_sb, in_=bq_d[:].rearrange("(p one) -> p one", one=1)
            )
            bk_sb = const.tile([C8, 1], F32, tag="bk")
            nc.scalar.dma_start(
                out=bk_sb, in_=bk_d[:].rearrange("(p one) -> p one", one=1)
            )
            bv2_sb = const.tile([CH, P], F32, tag="bv2")
            nc.scalar.dma_start(
                out=bv2_sb, in_=bv_d[:].rearrange("(ch p) -> ch p", p=P)
            )
            gamma_ap = gamma_d[:]
            gamma_sb = const.tile([P, 1], F32, tag="gamma")
            nc.scalar.dma_start(
                out=gamma_sb,
                in_=bass.AP(
                    tensor=gamma_ap.tensor, offset=gamma_ap.offset,
                    ap=[[0, P], gamma_ap.ap[0]],
                ),
            )
            # x loads in quarters: ch0 on the sync queue, ch1 on gpsimd,
            # interleaved with the weight loads so early windows land first.
            NQT = NH // 2
            xq = [xpool.tile([P, CH, NQT], F32, tag=f"xq{i}", name=f"xq{i}")
                  for i in range(4)]
            xbq = [xpool.tile([P, CH, NQT], BF16, tag=f"xbq{i}", name=f"xbq{i}")
                   for i in range(4)]
            nc.sync.dma_start(out=xq[0][:, 0, :], in_=x_d[0:P, 0:NQT])
            nc.gpsimd.dma_start(out=xq[0][:, 1, :], in_=x_d[P : 2 * P, 0:NQT])
            wq_stage = const.tile([C8, C], F32, tag="wqs")
            nc.gpsimd.dma_start(out=wq_stage, in_=wq_d[:, :])
            wk_stage = const.tile([C8, C], F32, tag="wks")
            nc.gpsimd.dma_start(out=wk_stage, in_=wk_d[:, :])
            wv_stage = const.tile([P, CH, C], F32, tag="wvs")
            nc.gpsimd.dma_start(
                out=wv_stage, in_=wv_d[:, :].rearrange("(a p) c -> p a c", p=P)
            )
            for i in range(1, 4):
                lo = i * NQT
                nc.sync.dma_start(out=xq[i][:, 0, :], in_=x_d[0:P, lo : lo + NQT])
                nc.gpsimd.dma_start(
                    out=xq[i][:, 1, :], in_=x_d[P : 2 * P, lo : lo + NQT]
                )
            gbv = const.tile([P, CH], F32, tag="gbv")

            def x_win(iw):  # fp32 residual slice [P, CH, IW]
                i = (iw * IW) // NQT
                off = iw * IW - i * NQT
                return xq[i][:, :, off : off + IW]

            def xb_win(iw):  # bf16 slice [P, CH, IW]
                i = (iw * IW) // NQT
                off = iw * IW - i * NQT
                return xbq[i][:, :, off : off + IW]

            def emit_xcast(iw):
                nc.vector.tensor_copy(xb_win(iw), x_win(iw))

            # ------------- weight transposes (bf16) -------------
            # wqkt[c, ch, 0:32] = wq^T chunk, wqkt[c, ch, 32:64] = wk^T chunk,
            # so one matmul chain projects q and k together.
            wqkt = const.tile([P, CH, 2 * C8], BF16, tag="wqkt")
            for ch in range(CH):
                ps_tqk = pe_ps.tile([P, 2 * C8], F32, tag="peps", name=f"ps_tqk{ch}")
                nc.tensor.transpose(
                    ps_tqk[:, 0:C8], wq_stage[:, bass.ts(ch, P)], ident[:C8, :C8]
                )
                nc.tensor.transpose(
                    ps_tqk[:, C8 : 2 * C8], wk_stage[:, bass.ts(ch, P)],
                    ident[:C8, :C8]
                )
                nc.vector.tensor_copy(wqkt[:, ch, :], ps_tqk)

            emit_xcast(0)

            # wvt[c, ci, o] = Wv[o, ci*128+c], bf16 (gamma is folded into
            # the rowsum stationary ones_g = (1/gamma) * ones instead)
            wvt = const.tile([P, CH, C], BF16, tag="wvt")
            for ci in range(CH):
                for oi in range(CH):
                    pool, ptag = (pe_ps, "peps") if oi == 0 else (av_ps, "avps")
                    ps_tv = pool.tile([P, P], F32, tag=ptag, name=f"ps_tv{ci}{oi}")
                    nc.tensor.transpose(
                        ps_tv, wv_stage[:, oi, bass.ts(ci, P)], ident
                    )
                    nc.vector.tensor_copy(wvt[:, ci, bass.ts(oi, P)], ps_tv)

            # ---------------- projections ----------------
            # q4/k4: [64, n] bf16, q/k replicated x2 across partition groups
            # for the 2-way row-packed QK matmuls.  One fused chain projects
            # q and k together into qk_s; idle DMA queues do the replication.
            qk_s = qkpool.tile([2 * C8, n], BF16, tag="qks")
            q4 = qkpool.tile([2 * C8, n], BF16, tag="q4")
            k4 = qkpool.tile([2 * C8, n], BF16, tag="k4")
            bqk_sb = const.tile([2 * C8, 1], F32, tag="bqk")

            def emit_qkproj(iw):
                win = bass.ts(iw, IW)
                xbw = xb_win(iw)
                ps_qk = pe_ps.tile([P, IW], F32, tag="peps", name=f"ps_qk_{iw}")
                for ch in range(CH):
                    nc.tensor.matmul(
                        ps_qk[0 : 2 * C8, :], wqkt[:, ch, :], xbw[:, ch, :],
                        start=(ch == 0), stop=(ch == CH - 1),
                    )
                nc.vector.tensor_scalar_add(
                    qk_s[:, win], ps_qk[0 : 2 * C8, :], bqk_sb
                )
                # replicate into the packed layouts on idle DMA queues
                nc.sync.dma_start(out=q4[0:C8, win], in_=qk_s[0:C8, win])
                nc.gpsimd.dma_start(out=q4[C8 : 2 * C8, win], in_=qk_s[0:C8, win])
                nc.sync.dma_start(out=k4[0:C8, win], in_=qk_s[C8 : 2 * C8, win])
                nc.gpsimd.dma_start(
                    out=k4[C8 : 2 * C8, win], in_=qk_s[C8 : 2 * C8, win]
                )

            # vT per key tile: vt[jt][p, c] = gamma * (Wv x)[c, jt*128+p], bf16
            vt = [None] * JT

            def emit_vproj(jt, cast_on_act=False):
                vtt = vtpool.tile([P, C], BF16, tag=f"vt{jt}", name=f"vt{jt}")
                ps_v = pe_ps.tile([P, C], F32, tag="peps", name=f"ps_v{jt}")
                iww, off = (jt * P) // IW, (jt * P) % IW
                xbw = xb_win(iww)
                for ch in range(CH):
                    nc.tensor.matmul(
                        ps_v,
                        xbw[:, ch, off : off + P],
                        wvt[:, ch, :],
                        start=(ch == 0), stop=(ch == CH - 1),
                    )
                if cast_on_act:
                    nc.scalar.copy(vtt, ps_v)
                else:
                    nc.vector.tensor_copy(vtt, ps_v)
                vt[jt] = vtt

            nc.vector.tensor_copy(bqk_sb[0:C8, :], bq_sb)
            nc.vector.tensor_copy(bqk_sb[C8 : 2 * C8, :], bk_sb)
            emit_qkproj(0)
            for jt in range(4):
                emit_vproj(jt)
            emit_xcast(1)
            emit_qkproj(1)
            # gbv = gamma * bv via on-chip transpose of the fast-shape load
            ps_bv = pe_ps.tile([P, CH], F32, tag="peps", name="ps_bv")
            nc.tensor.transpose(ps_bv, bv2_sb, ident[:CH, :CH])
            nc.vector.tensor_scalar_mul(gbv, ps_bv, gamma_sb)
            # rowsum stationary carries 1/gamma so rinv = gamma / rowsum
            giv = const.tile([P, 1], F32, tag="giv")
            nc.vector.reciprocal(giv, gamma_sb)
            ones_g = const.tile([P, P], BF16, tag="onesg")
            nc.vector.tensor_scalar_mul(ones_g, ones_bf, giv)
            for jt in range(4, 8):
                emit_vproj(jt)
            vjt_late = list(range(8, JT))
            qk_late = list(range(2, NW))
            xc_late = list(range(2, NW))

            # ---------------- main pipeline ----------------
            state = {}
            pse_hist = {}

            def emit_group(g):
                iw, gg = divmod(g, GPW)
                win = bass.ts(iw, IW)
                if gg == 0:
                    state[iw] = {
                        "av": av_ps.tile(
                            [P, CH, IW], F32, tag="avps", name=f"av_{iw}"
                        ),
                        "acc": accpool.tile(
                            [P, 2, IW], BF16, tag="acc", name=f"acc_{iw}"
                        ),
                    }
                ps_e = pe_ps.tile([P, 2, IW], F32, tag="peps", name=f"ps_e{g}")
                pse_hist[g] = ps_e
                pse_hist.pop(g - 4, None)
                for m in range(2):
                    jt = 2 * gg + m
                    nc.tensor.matmul(
                        ps_e[:, m, :],
                        k4[m * C8 : (m + 1) * C8, bass.ts(jt, P)],
                        q4[m * C8 : (m + 1) * C8, win],
                        start=True, stop=True,
                        tile_position=(m * C8, 0),
                    )
                pt = ptpool.tile([P, 2, IW], BF16, tag="pt", name=f"pt{g}")
                nc.scalar.activation(pt, ps_e, mybir.ActivationFunctionType.Exp)
                acc = state[iw]["acc"]
                if gg == 0:
                    nc.vector.tensor_copy(acc, pt)
                else:
                    nc.vector.tensor_add(acc, acc, pt)
                return pt

            def emit_av(g, pt):
                iw, gg = divmod(g, GPW)
                av = state[iw]["av"]
                for m in range(2):
                    jt = 2 * gg + m
                    for ch in range(CH):
                        nc.tensor.matmul(
                            av[:, ch, :],
                            vt[jt][:, bass.ts(ch, P)],
                            pt[:, m, :],
                            start=(gg == 0 and m == 0),
                            stop=(gg == GPW - 1 and m == 1),
                            skip_group_check=True,
                        )

            def emit_epilogue(iw, cur_g):
                st = state.pop(iw)
                acc, av = st["acc"], st["av"]
                win = bass.ts(iw, IW)
                # rowsum goes into the previous group's ps_e tile: its exp
                # has already drained it, and the tile's next QK writer is
                # two groups out, past the reciprocal read below.  Reusing a
                # live tile (instead of allocating) keeps the pe_ps rotation
                # parity intact, so no group ever single-buffers.
                host = max(k for k in pse_hist if k <= cur_g - 1)
                ps_r = pse_hist[host][:, 0, :]
                for s in range(2):
                    nc.tensor.matmul(
                        ps_r, ones_g, acc[:, s, :],
                        start=(s == 0), stop=(s == 1),
                    )
                rinv = smallwork.tile([P, IW], F32, tag="rinv", name=f"rinv{iw}")
                nc.vector.reciprocal_approx_fast(rinv, ps_r)
                xw = x_win(iw)
                for ch in range(CH):
                    o_sb = outpool.tile([P, IW], F32, tag="osb", name=f"osb{ch}_{iw}")
                    nc.vector.tensor_mul(o_sb, av[:, ch, :], rinv)
                    nc.vector.scalar_tensor_tensor(
                        out=o_sb, in0=o_sb, scalar=gbv[:, ch : ch + 1],
                        in1=xw[:, ch, :],
                        op0=mybir.AluOpType.add, op1=mybir.AluOpType.add,
                    )
                    eng = nc.sync if ch == 0 else nc.gpsimd
                    eng.dma_start(
                        out=out_d[ch * P : (ch + 1) * P, win], in_=o_sb
                    )

            pts = [None] * NG
            for g in range(NG + 1):
                if g < NG:
                    for _ in range(2):
                        if xc_late:
                            emit_xcast(xc_late.pop(0))
                    pts[g] = emit_group(g)
                if g > 0:
                    emit_av(g - 1, pts[g - 1])
                    pts[g - 1] = None
                if g < NG:
                    # drips go AFTER emit_av so their psum-rotation steals
                    # never stall the PE ahead of the AV matmuls
                    for _ in range(2):
                        if qk_late:
                            emit_qkproj(qk_late.pop(0))
                    for dv in range(3):
                        if vjt_late:
                            emit_vproj(vjt_late.pop(0), cast_on_act=(dv == 0))
                if g > 0 and g - 1 >= GPW + 1 and (g - 1) % GPW == 1:
                    emit_epilogue((g - 1) // GPW - 1, g)
            emit_epilogue(NW - 1, NG)

    nc.finalize()
    return nc


_NC_CACHE: dict[int, bass.Bass] = {}


def _get_nc(n: int) -> bass.Bass:
    if n not in _NC_CACHE:
        _NC_CACHE[n] = build_attention_nc(n)
    return _NC_CACHE[n]


def kernel(x, Wq, bq, Wk, bk, Wv, bv, gamma):
    B, c, h, w = x.shape
    n = h * w
    assert B == 8 and c == C
    nc = _get_nc(n)
    xf = np.ascontiguousarray(np.asarray(x, dtype=np.float32).reshape(B, c, n))
    common = {
        "Wq": np.ascontiguousarray(np.asarray(Wq, dtype=np.float32)),
        "bq": np.ascontiguousarray(np.asarray(bq, dtype=np.float32)),
        "Wk": np.ascontiguousarray(np.asarray(Wk, dtype=np.float32)),
        "bk": np.ascontiguousarray(np.asarray(bk, dtype=np.float32)),
        "Wv": np.ascontiguousarray(np.asarray(Wv, dtype=np.float32)),
        "bv": np.ascontiguousarray(np.asarray(bv, dtype=np.float32)),
        "gamma": np.ascontiguousarray(np.asarray(gamma, dtype=np.float32)),
    }
    in_maps = [{"x": xf[b], **common} for b in range(B)]
    res = run_bass_kernel_spmd(nc, in_maps, core_ids=list(range(B)))
    out = np.stack([res.results[b]["out"].reshape(c, h, w) for b in range(B)])
    return out.astype(np.float32)


# revision 12
# speedup vs baseline: 1.0451x; 1.0165x over previous
"""Self-contained Trainium2 Bass kernel for the AttentionBlock problem.

Shapes (hardcoded): x [8, 256, 64, 64] fp32, Wq/Wk [32, 256], bq/bk [32],
Wv [256, 256], bv [256], gamma [1].

Sharding: data-parallel over batch - each of the 8 NeuronCores computes the
full 4096x4096 attention for one batch element. No collectives.

Per-core algorithm (C=256, C8=32, N=4096), fully SBUF-resident.
Pipeline unit = "group": 2 key tiles x 512-query window (N=512 moving
operands keep every matmul streaming-bound; shorter moving lengths are
LDWEIGHTS-bound at ~131 ns/matmul).
  QK   2 row-packed K=32 bf16 matmuls (tile_position 0/32) -> one psum
       tile [128, 2, 512] (2 banks, double buffered)
  exp  one FD=1024 ACT instruction psum -> pt bf16
  acc  acc += pt on DVE (bf16 2x) - per-partition rowsum partials
  AV   4 bf16 matmuls (2 jt x 2 ch) accumulate v.T@p into av [128,2,512]
Per 512-query window (16 groups): rowsum = ones.T @ acc (2 matmuls into a
psum tile STOLEN from the QK pool's rotation - the buffer was just drained
by exp, so no extra banks), rinv = recip(rowsum), out = av*rinv +
(gamma*bv + x) with the epilogue reading av straight out of PSUM.  gamma
is folded into Wv.  The q/k projections run as one fused chain (wq|wk in
one 64-wide stationary), and the x2 replication needed for the row-packed
QK is done by SBUF-to-SBUF DMAs on otherwise-idle queues.  All projection
/ rowsum psum scratch is allocated from the QK pool's tag rotation, so
PSUM is exactly 8 banks: QK 2x2 + AV accumulators 2x2.
"""

import sys

import numpy as np

if "/opt/trn_rl_repo" not in sys.path:
    sys.path.insert(0, "/opt/trn_rl_repo")

import concourse.bass as bass
import concourse.bacc as bacc
import concourse.tile as tile
from concourse import mybir
from concourse.bass_utils import run_bass_kernel_spmd
from concourse.masks import make_identity

F32 = mybir.dt.float32
BF16 = mybir.dt.bfloat16

C = 256
C8 = 32
P = 128
CH = C // P  # 2 channel chunks
IW = 512     # query-window size


def build_attention_nc(n: int = 4096) -> bass.Bass:
    """Build the single-core Bass program (SPMD across 8 cores)."""
    assert n % IW == 0
    NW = n // IW        # query windows (8)
    JT = n // P         # key tiles (32)
    GPW = JT // 2       # groups per window (16)
    NG = NW * GPW       # total groups (128)
    NH = n // 2         # half of the token dim (x loaded as 2 halves)

    nc = bacc.Bacc("TRN2", target_bir_lowering=False)
    x_d = nc.declare_dram_parameter("x", [C, n], F32, isOutput=False)
    wq_d = nc.declare_dram_parameter("Wq", [C8, C], F32, isOutput=False)
    bq_d = nc.declare_dram_parameter("bq", [C8], F32, isOutput=False)
    wk_d = nc.declare_dram_parameter("Wk", [C8, C], F32, isOutput=False)
    bk_d = nc.declare_dram_parameter("bk", [C8], F32, isOutput=False)
    wv_d = nc.declare_dram_parameter("Wv", [C, C], F32, isOutput=False)
    bv_d = nc.declare_dram_parameter("bv", [C], F32, isOutput=False)
    gamma_d = nc.declare_dram_parameter("gamma", [1], F32, isOutput=False)
    out_d = nc.declare_dram_parameter("out", [C, n], F32, isOutput=True)

    with tile.TileContext(nc) as tc:
        with (
            tc.tile_pool(name="const", bufs=1) as const,
            tc.tile_pool(name="xpool", bufs=1) as xpool,
            tc.tile_pool(name="qkpool", bufs=1) as qkpool,
            tc.tile_pool(name="vtpool", bufs=1) as vtpool,
            tc.tile_pool(name="ptpool", bufs=3) as ptpool,
            tc.tile_pool(name="accpool", bufs=2) as accpool,
            tc.tile_pool(name="smallwork", bufs=4) as smallwork,
            tc.tile_pool(name="outpool", bufs=6) as outpool,
            tc.tile_pool(name="pe_ps", bufs=2, space="PSUM") as pe_ps,  # 2x2 banks
            tc.tile_pool(name="av_ps", bufs=2, space="PSUM") as av_ps,  # 2x2 banks
        ):
            # ---------------- setup: loads ----------------
            # warm the ACT exp table immediately
            warm_in = const.tile([P, 1], F32, tag="warmin")
            nc.gpsimd.memset(warm_in, 0.0)
            warm_out = const.tile([P, 1], F32, tag="warmout")
            nc.scalar.activation(warm_out, warm_in, mybir.ActivationFunctionType.Exp)

            ident = const.tile([P, P], F32, tag="ident")
            make_identity(nc, ident)

            ones_bf = const.tile([P, P], BF16, tag="ones")
            nc.vector.memset(ones_bf, 1.0)

            # tiny bias loads go FIRST so their packets clear the DMA rings
            # before the x flood; bq/bk are [32,1] (32 packets), bv is loaded
            # [2,128] (2 packets) and transposed on-chip later.  The gamma
            # partition-broadcast (128 tiny packets) is slow but nothing on
            # the critical path consumes it.
            bq_sb = const.tile([C8, 1], F32, tag="bq")
            nc.scalar.dma_start(
                out=bq_sb, in_=bq_d[:].rearrange("(p one) -> p one", one=1)
            )
            bk_sb = const.tile([C8, 1], F32, tag="bk")
            nc.scalar.dma_start(
                out=bk_sb, in_=bk_d[:].rearrange("(p one) -> p one", one=1)
            )
            bv2_sb = const.tile([CH, P], F32, tag="bv2")
            nc.scalar.dma_start(
                out=bv2_sb, in_=bv_d[:].rearrange("(ch p) -> ch p", p=P)
            )
            gamma_ap = gamma_d[:]
            gamma_sb = const.tile([P, 1], F32, tag="gamma")
            nc.scalar.dma_start(
                out=gamma_sb,
                in_=bass.AP(
                    tensor=gamma_ap.tensor, offset=gamma_ap.offset,
                    ap=[[0, P], gamma_ap.ap[0]],
                ),
            )
            # x loads in quarters: ch0 on the sync queue, ch1 on gpsimd,
            # interleaved with the weight loads so early windows land first.
            NQT = NH // 2
            xq = [xpool.tile([P, CH, NQT], F32, tag=f"xq{i}", name=f"xq{i}")
                  for i in range(4)]
            xbq = [xpool.tile([P, CH, NQT], BF16, tag=f"xbq{i}", name=f"xbq{i}")
                   for i in range(4)]
            wq_stage = const.tile([C8, C], F32, tag="wqs")
            nc.sync.dma_start(out=wq_stage, in_=wq_d[:, :])
            wk_stage = const.tile([C8, C], F32, tag="wks")
            nc.sync.dma_start(out=wk_stage, in_=wk_d[:, :])
            wv_stage = const.tile([P, CH, C], F32, tag="wvs")
            nc.sync.dma_start(
                out=wv_stage, in_=wv_d[:, :].rearrange("(a p) c -> p a c", p=P)
            )
            for i in range(4):
                lo = i * NQT
                nc.sync.dma_start(out=xq[i][:, 0, :], in_=x_d[0:P, lo : lo + NQT])
                nc.gpsimd.dma_start(
                    out=xq[i][:, 1, :], in_=x_d[P : 2 * P, lo : lo + NQT]
                )
            gbv = const.tile([P, CH], F32, tag="gbv")

            def x_win(iw):  # fp32 residual slice [P, CH, IW]
                i = (iw * IW) // NQT
                off = iw * IW - i * NQT
                return xq[i][:, :, off : off + IW]

            def xb_win(iw):  # bf16 slice [P, CH, IW]
                i = (iw * IW) // NQT
                off = iw * IW - i * NQT
                return xbq[i][:, :, off : off + IW]

            def emit_xcast(iw):
                nc.vector.tensor_copy(xb_win(iw), x_win(iw))

            # ------------- weight transposes (bf16) -------------
            # wqkt[c, ch, 0:32] = wq^T chunk, wqkt[c, ch, 32:64] = wk^T chunk,
            # so one matmul chain projects q and k together.
            wqkt = const.tile([P, CH, 2 * C8], BF16, tag="wqkt")
            for ch in range(CH):
                ps_tqk = pe_ps.tile([P, 2 * C8], F32, tag="peps", name=f"ps_tqk{ch}")
                nc.tensor.transpose(
                    ps_tqk[:, 0:C8], wq_stage[:, bass.ts(ch, P)], ident[:C8, :C8]
                )
                nc.tensor.transpose(
                    ps_tqk[:, C8 : 2 * C8], wk_stage[:, bass.ts(ch, P)],
                    ident[:C8, :C8]
                )
                nc.vector.tensor_copy(wqkt[:, ch, :], ps_tqk)

            emit_xcast(0)

            # wvt[c, ci, o] = Wv[o, ci*128+c], bf16 (gamma is folded into
            # the rowsum stationary ones_g = (1/gamma) * ones instead)
            wvt = const.tile([P, CH, C], BF16, tag="wvt")
            for ci in range(CH):
                for oi in range(CH):
                    pool, ptag = (pe_ps, "peps") if oi == 0 else (av_ps, "avps")
                    ps_tv = pool.tile([P, P], F32, tag=ptag, name=f"ps_tv{ci}{oi}")
                    nc.tensor.transpose(
                        ps_tv, wv_stage[:, oi, bass.ts(ci, P)], ident
                    )
                    nc.vector.tensor_copy(wvt[:, ci, bass.ts(oi, P)], ps_tv)

            # ---------------- projections ----------------
            # q4/k4: [64, n] bf16, q/k replicated x2 across partition groups
            # for the 2-way row-packed QK matmuls.  One fused chain projects
            # q and k together into qk_s; idle DMA queues do the replication.
            qk_s = qkpool.tile([2 * C8, n], BF16, tag="qks")
            q4 = qkpool.tile([2 * C8, n], BF16, tag="q4")
            k4 = qkpool.tile([2 * C8, n], BF16, tag="k4")
            bqk_sb = const.tile([2 * C8, 1], F32, tag="bqk")

            def emit_qkproj(iw):
                win = bass.ts(iw, IW)
                xbw = xb_win(iw)
                ps_qk = pe_ps.tile([P, IW], F32, tag="peps", name=f"ps_qk_{iw}")
                for ch in range(CH):
                    nc.tensor.matmul(
                        ps_qk[0 : 2 * C8, :], wqkt[:, ch, :], xbw[:, ch, :],
                        start=(ch == 0), stop=(ch == CH - 1),
                    )
                nc.scalar.activation(
                    qk_s[:, win], ps_qk[0 : 2 * C8, :],
                    mybir.ActivationFunctionType.Identity,
                    bias=bqk_sb, scale=1.0,
                )
                # replicate into the packed layouts on idle DMA queues
                nc.sync.dma_start(out=q4[0:C8, win], in_=qk_s[0:C8, win])
                nc.gpsimd.dma_start(out=q4[C8 : 2 * C8, win], in_=qk_s[0:C8, win])
                nc.sync.dma_start(out=k4[0:C8, win], in_=qk_s[C8 : 2 * C8, win])
                nc.gpsimd.dma_start(
                    out=k4[C8 : 2 * C8, win], in_=qk_s[C8 : 2 * C8, win]
                )

            # vT per key tile: vt[jt][p, c] = gamma * (Wv x)[c, jt*128+p], bf16
            vt = [None] * JT

            def emit_vproj(jt, cast_on_act=False):
                vtt = vtpool.tile([P, C], BF16, tag=f"vt{jt}", name=f"vt{jt}")
                ps_v = pe_ps.tile([P, C], F32, tag="peps", name=f"ps_v{jt}")
                iww, off = (jt * P) // IW, (jt * P) % IW
                xbw = xb_win(iww)
                for ch in range(CH):
                    nc.tensor.matmul(
                        ps_v,
                        xbw[:, ch, off : off + P],
                        wvt[:, ch, :],
                        start=(ch == 0), stop=(ch == CH - 1),
                    )
                if cast_on_act:
                    nc.scalar.copy(vtt, ps_v)
                else:
                    nc.vector.tensor_copy(vtt, ps_v)
                vt[jt] = vtt

            nc.vector.tensor_copy(bqk_sb[0:C8, :], bq_sb)
            nc.vector.tensor_copy(bqk_sb[C8 : 2 * C8, :], bk_sb)
            emit_xcast(2)
            emit_qkproj(0)
            for jt in range(4):
                emit_vproj(jt)
            emit_xcast(1)
            emit_qkproj(1)
            # gbv = gamma * bv via on-chip transpose of the fast-shape load
            ps_bv = pe_ps.tile([P, CH], F32, tag="peps", name="ps_bv")
            nc.tensor.transpose(ps_bv, bv2_sb, ident[:CH, :CH])
            nc.vector.tensor_scalar_mul(gbv, ps_bv, gamma_sb)
            # rowsum stationary carries 1/gamma so rinv = gamma / rowsum
            giv = const.tile([P, 1], F32, tag="giv")
            nc.vector.reciprocal(giv, gamma_sb)
            ones_g = const.tile([P, P], BF16, tag="onesg")
            nc.vector.tensor_scalar_mul(ones_g, ones_bf, giv)
            for jt in range(4, 8):
                emit_vproj(jt)
            emit_xcast(3)
            vjt_late = list(range(8, JT))
            qk_late = list(range(2, NW))
            xc_late = list(range(4, NW))

            # ---------------- main pipeline ----------------
            state = {}
            pse_hist = {}

            def emit_group(g):
                iw, gg = divmod(g, GPW)
                win = bass.ts(iw, IW)
                if gg == 0:
                    state[iw] = {
                        "av": av_ps.tile(
                            [P, CH, IW], F32, tag="avps", name=f"av_{iw}"
                        ),
                        "acc": accpool.tile(
                            [P, 2, IW], BF16, tag="acc", name=f"acc_{iw}"
                        ),
                    }
                ps_e = pe_ps.tile([P, 2, IW], F32, tag="peps", name=f"ps_e{g}")
                pse_hist[g] = ps_e
                pse_hist.pop(g - 4, None)
                for m in range(2):
                    jt = 2 * gg + m
                    nc.tensor.matmul(
                        ps_e[:, m, :],
                        k4[m * C8 : (m + 1) * C8, bass.ts(jt, P)],
                        q4[m * C8 : (m + 1) * C8, win],
                        start=True, stop=True,
                        tile_position=(m * C8, 0),
                    )
                pt = ptpool.tile([P, 2, IW], BF16, tag="pt", name=f"pt{g}")
                nc.scalar.activation(pt, ps_e, mybir.ActivationFunctionType.Exp)
                acc = state[iw]["acc"]
                if gg == 0:
                    nc.vector.tensor_copy(acc, pt)
                else:
                    nc.vector.tensor_add(acc, acc, pt)
                return pt

            def emit_av(g, pt):
                iw, gg = divmod(g, GPW)
                av = state[iw]["av"]
                for m in range(2):
                    jt = 2 * gg + m
                    for ch in range(CH):
                        nc.tensor.matmul(
                            av[:, ch, :],
                            vt[jt][:, bass.ts(ch, P)],
                            pt[:, m, :],
                            start=(gg == 0 and m == 0),
                            stop=(gg == GPW - 1 and m == 1),
                            skip_group_check=True,
                        )

            def emit_epilogue(iw, cur_g):
                st = state.pop(iw)
                acc, av = st["acc"], st["av"]
                win = bass.ts(iw, IW)
                # rowsum goes into the previous group's ps_e tile: its exp
                # has already drained it, and the tile's next QK writer is
                # two groups out, past the reciprocal read below.  Reusing a
                # live tile (instead of allocating) keeps the pe_ps rotation
                # parity intact, so no group ever single-buffers.
                host = max(k for k in pse_hist if k <= cur_g - 1)
                ps_r = pse_hist[host][:, 0, :]
                for s in range(2):
                    nc.tensor.matmul(
                        ps_r, ones_g, acc[:, s, :],
                        start=(s == 0), stop=(s == 1),
                    )
                rinv = smallwork.tile([P, IW], F32, tag="rinv", name=f"rinv{iw}")
                nc.vector.reciprocal_approx_fast(rinv, ps_r)
                xw = x_win(iw)
                for ch in range(CH):
                    o_sb = outpool.tile([P, IW], F32, tag="osb", name=f"osb{ch}_{iw}")
                    nc.vector.tensor_mul(o_sb, av[:, ch, :], rinv)
                    nc.vector.scalar_tensor_tensor(
                        out=o_sb, in0=o_sb, scalar=gbv[:, ch : ch + 1],
                        in1=xw[:, ch, :],
                        op0=mybir.AluOpType.add, op1=mybir.AluOpType.add,
                    )
                    eng = nc.sync if ch == 0 else nc.gpsimd
                    eng.dma_start(
                        out=out_d[ch * P : (ch + 1) * P, win], in_=o_sb
                    )

            pts = [None] * NG
            for g in range(NG + 1):
                if g < NG:
                    if xc_late:
                        emit_xcast(xc_late.pop(0))
                    pts[g] = emit_group(g)
                if g > 0:
                    emit_av(g - 1, pts[g - 1])
                    pts[g - 1] = None
                if g < NG:
                    # drips go AFTER emit_av so their psum-rotation steals
                    # never stall the PE ahead of the AV matmuls
                    if qk_late:
                        emit_qkproj(qk_late.pop(0))
                    for _ in range(3):
                        if vjt_late:
                            emit_vproj(vjt_late.pop(0))
                if g > 0 and g - 1 >= GPW + 1 and (g - 1) % GPW == 1:
                    emit_epilogue((g - 1) // GPW - 1, g)
            emit_epilogue(NW - 1, NG)

    nc.finalize()
    return nc


_NC_CACHE: dict[int, bass.Bass] = {}


def _get_nc(n: int) -> bass.Bass:
    if n not in _NC_CACHE:
        _NC_CACHE[n] = build_attention_nc(n)
    return _NC_CACHE[n]


def kernel(x, Wq, bq, Wk, bk, Wv, bv, gamma):
    B, c, h, w = x.shape
    n = h * w
    assert B == 8 and c == C
    nc = _get_nc(n)
    xf = np.ascontiguousarray(np.asarray(x, dtype=np.float32).reshape(B, c, n))
    common = {
        "Wq": np.ascontiguousarray(np.asarray(Wq, dtype=np.float32)),
        "bq": np.ascontiguousarray(np.asarray(bq, dtype=np.float32)),
        "Wk": np.ascontiguousarray(np.asarray(Wk, dtype=np.float32)),
        "bk": np.ascontiguousarray(np.asarray(bk, dtype=np.float32)),
        "Wv": np.ascontiguousarray(np.asarray(Wv, dtype=np.float32)),
        "bv": np.ascontiguousarray(np.asarray(bv, dtype=np.float32)),
        "gamma": np.ascontiguousarray(np.asarray(gamma, dtype=np.float32)),
    }
    in_maps = [{"x": xf[b], **common} for b in range(B)]
    res = run_bass_kernel_spmd(nc, in_maps, core_ids=list(range(B)))
    out = np.stack([res.results[b]["out"].reshape(c, h, w) for b in range(B)])
    return out.astype(np.float32)


# revision 13
# speedup vs baseline: 1.0515x; 1.0061x over previous
"""Self-contained Trainium2 Bass kernel for the AttentionBlock problem.

Shapes (hardcoded): x [8, 256, 64, 64] fp32, Wq/Wk [32, 256], bq/bk [32],
Wv [256, 256], bv [256], gamma [1].

Sharding: data-parallel over batch - each of the 8 NeuronCores computes the
full 4096x4096 attention for one batch element. No collectives.

Per-core algorithm (C=256, C8=32, N=4096), fully SBUF-resident.
Pipeline unit = "group": 2 key tiles x 512-query window (N=512 moving
operands keep every matmul streaming-bound; shorter moving lengths are
LDWEIGHTS-bound at ~131 ns/matmul).
  QK   2 row-packed K=32 bf16 matmuls (tile_position 0/32) -> one psum
       tile [128, 2, 512] (2 banks, double buffered)
  exp  one FD=1024 ACT instruction psum -> pt bf16
  acc  acc += pt on DVE (bf16 2x) - per-partition rowsum partials
  AV   4 bf16 matmuls (2 jt x 2 ch) accumulate v.T@p into av [128,2,512]
Per 512-query window (16 groups): rowsum = ones.T @ acc (2 matmuls into a
psum tile STOLEN from the QK pool's rotation - the buffer was just drained
by exp, so no extra banks), rinv = recip(rowsum), out = av*rinv +
(gamma*bv + x) with the epilogue reading av straight out of PSUM.  gamma
is folded into Wv.  The q/k projections run as one fused chain (wq|wk in
one 64-wide stationary), and the x2 replication needed for the row-packed
QK is done by SBUF-to-SBUF DMAs on otherwise-idle queues.  All projection
/ rowsum psum scratch is allocated from the QK pool's tag rotation, so
PSUM is exactly 8 banks: QK 2x2 + AV accumulators 2x2.
"""

import sys

import numpy as np

if "/opt/trn_rl_repo" not in sys.path:
    sys.path.insert(0, "/opt/trn_rl_repo")

import concourse.bass as bass
import concourse.bacc as bacc
import concourse.tile as tile
from concourse import mybir
from concourse.bass_utils import run_bass_kernel_spmd
from concourse.masks import make_identity

F32 = mybir.dt.float32
BF16 = mybir.dt.bfloat16

C = 256
C8 = 32
P = 128
CH = C // P  # 2 channel chunks
IW = 512     # query-window size


def build_attention_nc(n: int = 4096) -> bass.Bass:
    """Build the single-core Bass program (SPMD across 8 cores)."""
    assert n % IW == 0
    NW = n // IW        # query windows (8)
    JT = n // P         # key tiles (32)
    GPW = JT // 2       # groups per window (16)
    NG = NW * GPW       # total groups (128)
    NH = n // 2         # half of the token dim (x loaded as 2 halves)

    nc = bacc.Bacc("TRN2", target_bir_lowering=False)
    x_d = nc.declare_dram_parameter("x", [C, n], F32, isOutput=False)
    wq_d = nc.declare_dram_parameter("Wq", [C8, C], F32, isOutput=False)
    bq_d = nc.declare_dram_parameter("bq", [C8], F32, isOutput=False)
    wk_d = nc.declare_dram_parameter("Wk", [C8, C], F32, isOutput=False)
    bk_d = nc.declare_dram_parameter("bk", [C8], F32, isOutput=False)
    wv_d = nc.declare_dram_parameter("Wv", [C, C], F32, isOutput=False)
    bv_d = nc.declare_dram_parameter("bv", [C], F32, isOutput=False)
    gamma_d = nc.declare_dram_parameter("gamma", [1], F32, isOutput=False)
    out_d = nc.declare_dram_parameter("out", [C, n], F32, isOutput=True)

    with tile.TileContext(nc) as tc:
        with (
            tc.tile_pool(name="const", bufs=1) as const,
            tc.tile_pool(name="xpool", bufs=1) as xpool,
            tc.tile_pool(name="qkpool", bufs=1) as qkpool,
            tc.tile_pool(name="vtpool", bufs=1) as vtpool,
            tc.tile_pool(name="ptpool", bufs=3) as ptpool,
            tc.tile_pool(name="accpool", bufs=2) as accpool,
            tc.tile_pool(name="smallwork", bufs=4) as smallwork,
            tc.tile_pool(name="outpool", bufs=6) as outpool,
            tc.tile_pool(name="pe_ps", bufs=2, space="PSUM") as pe_ps,  # 2x2 banks
            tc.tile_pool(name="av_ps", bufs=2, space="PSUM") as av_ps,  # 2x2 banks
        ):
            # ---------------- setup: loads ----------------
            # warm the ACT exp table immediately
            warm_in = const.tile([P, 1], F32, tag="warmin")
            nc.gpsimd.memset(warm_in, 0.0)
            warm_out = const.tile([P, 1], F32, tag="warmout")
            nc.scalar.activation(warm_out, warm_in, mybir.ActivationFunctionType.Exp)

            ident = const.tile([P, P], F32, tag="ident")
            make_identity(nc, ident)

            ones_bf = const.tile([P, P], BF16, tag="ones")
            nc.vector.memset(ones_bf, 1.0)

            # x loads in quarters.  Both HWDGE queues (sync + scalar) carry
            # them - the gpsimd SWDGE path has ~10us transfer latency and is
            # avoided for anything startup-critical.  Weights go first on
            # sync (they gate the transposes); the early x ch1 quarters ride
            # the scalar queue which is otherwise idle until the first exp.
            NQT = NH // 2
            xq = [xpool.tile([P, CH, NQT], F32, tag=f"xq{i}", name=f"xq{i}")
                  for i in range(4)]
            xbq = [xpool.tile([P, CH, NQT], BF16, tag=f"xbq{i}", name=f"xbq{i}")
                   for i in range(4)]
            nc.scalar.dma_start(out=xq[0][:, 1, :], in_=x_d[P : 2 * P, 0:NQT])
            nc.scalar.dma_start(
                out=xq[1][:, 1, :], in_=x_d[P : 2 * P, NQT : 2 * NQT]
            )
            wq_stage = const.tile([C8, C], F32, tag="wqs")
            nc.sync.dma_start(out=wq_stage, in_=wq_d[:, :])
            wk_stage = const.tile([C8, C], F32, tag="wks")
            nc.sync.dma_start(out=wk_stage, in_=wk_d[:, :])
            wv_stage = const.tile([P, CH, C], F32, tag="wvs")
            nc.sync.dma_start(
                out=wv_stage, in_=wv_d[:, :].rearrange("(a p) c -> p a c", p=P)
            )
            bq_sb = const.tile([C8, 1], F32, tag="bq")
            nc.scalar.dma_start(
                out=bq_sb, in_=bq_d[:].rearrange("(p one) -> p one", one=1)
            )
            bk_sb = const.tile([C8, 1], F32, tag="bk")
            nc.scalar.dma_start(
                out=bk_sb, in_=bk_d[:].rearrange("(p one) -> p one", one=1)
            )
            bv2_sb = const.tile([CH, P], F32, tag="bv2")
            nc.scalar.dma_start(
                out=bv2_sb, in_=bv_d[:].rearrange("(ch p) -> ch p", p=P)
            )
            gamma_ap = gamma_d[:]
            gamma_sb = const.tile([P, 1], F32, tag="gamma")
            nc.scalar.dma_start(
                out=gamma_sb,
                in_=bass.AP(
                    tensor=gamma_ap.tensor, offset=gamma_ap.offset,
                    ap=[[0, P], gamma_ap.ap[0]],
                ),
            )
            for i in range(4):
                lo = i * NQT
                nc.sync.dma_start(out=xq[i][:, 0, :], in_=x_d[0:P, lo : lo + NQT])
            nc.sync.dma_start(
                out=xq[2][:, 1, :], in_=x_d[P : 2 * P, 2 * NQT : 3 * NQT]
            )
            nc.sync.dma_start(
                out=xq[3][:, 1, :], in_=x_d[P : 2 * P, 3 * NQT : 4 * NQT]
            )
            gbv = const.tile([P, CH], F32, tag="gbv")

            def x_win(iw):  # fp32 residual slice [P, CH, IW]
                i = (iw * IW) // NQT
                off = iw * IW - i * NQT
                return xq[i][:, :, off : off + IW]

            def xb_win(iw):  # bf16 slice [P, CH, IW]
                i = (iw * IW) // NQT
                off = iw * IW - i * NQT
                return xbq[i][:, :, off : off + IW]

            def emit_xcast(iw):
                nc.vector.tensor_copy(xb_win(iw), x_win(iw))

            # ------------- weight transposes (bf16) -------------
            # wqkt[c, ch, 0:32] = wq^T chunk, wqkt[c, ch, 32:64] = wk^T chunk,
            # so one matmul chain projects q and k together.
            wqkt = const.tile([P, CH, 2 * C8], BF16, tag="wqkt")
            for ch in range(CH):
                ps_tqk = pe_ps.tile([P, 2 * C8], F32, tag="peps", name=f"ps_tqk{ch}")
                nc.tensor.transpose(
                    ps_tqk[:, 0:C8], wq_stage[:, bass.ts(ch, P)], ident[:C8, :C8]
                )
                nc.tensor.transpose(
                    ps_tqk[:, C8 : 2 * C8], wk_stage[:, bass.ts(ch, P)],
                    ident[:C8, :C8]
                )
                nc.vector.tensor_copy(wqkt[:, ch, :], ps_tqk)

            emit_xcast(0)

            # wvt[c, ci, o] = Wv[o, ci*128+c], bf16 (gamma is folded into
            # the rowsum stationary ones_g = (1/gamma) * ones instead)
            wvt = const.tile([P, CH, C], BF16, tag="wvt")
            for ci in range(CH):
                for oi in range(CH):
                    pool, ptag = (pe_ps, "peps") if oi == 0 else (av_ps, "avps")
                    ps_tv = pool.tile([P, P], F32, tag=ptag, name=f"ps_tv{ci}{oi}")
                    nc.tensor.transpose(
                        ps_tv, wv_stage[:, oi, bass.ts(ci, P)], ident
                    )
                    nc.vector.tensor_copy(wvt[:, ci, bass.ts(oi, P)], ps_tv)

            # ---------------- projections ----------------
            # q4/k4: [64, n] bf16, q/k replicated x2 across partition groups
            # for the 2-way row-packed QK matmuls.  One fused chain projects
            # q and k together into qk_s; idle DMA queues do the replication.
            qk_s = qkpool.tile([2 * C8, n], BF16, tag="qks")
            q4 = qkpool.tile([2 * C8, n], BF16, tag="q4")
            k4 = qkpool.tile([2 * C8, n], BF16, tag="k4")
            bqk_sb = const.tile([2 * C8, 1], F32, tag="bqk")

            def emit_qkproj(iw):
                win = bass.ts(iw, IW)
                xbw = xb_win(iw)
                ps_qk = pe_ps.tile([P, IW], F32, tag="peps", name=f"ps_qk_{iw}")
                for ch in range(CH):
                    nc.tensor.matmul(
                        ps_qk[0 : 2 * C8, :], wqkt[:, ch, :], xbw[:, ch, :],
                        start=(ch == 0), stop=(ch == CH - 1),
                    )
                nc.scalar.activation(
                    qk_s[:, win], ps_qk[0 : 2 * C8, :],
                    mybir.ActivationFunctionType.Identity,
                    bias=bqk_sb, scale=1.0,
                )
                # replicate into the packed layouts on idle DMA queues
                nc.sync.dma_start(out=q4[0:C8, win], in_=qk_s[0:C8, win])
                nc.gpsimd.dma_start(out=q4[C8 : 2 * C8, win], in_=qk_s[0:C8, win])
                nc.sync.dma_start(out=k4[0:C8, win], in_=qk_s[C8 : 2 * C8, win])
                nc.gpsimd.dma_start(
                    out=k4[C8 : 2 * C8, win], in_=qk_s[C8 : 2 * C8, win]
                )

            # vT per key tile: vt[jt][p, c] = gamma * (Wv x)[c, jt*128+p], bf16
            vt = [None] * JT

            def emit_vproj(jt, cast_on_act=False):
                vtt = vtpool.tile([P, C], BF16, tag=f"vt{jt}", name=f"vt{jt}")
                ps_v = pe_ps.tile([P, C], F32, tag="peps", name=f"ps_v{jt}")
                iww, off = (jt * P) // IW, (jt * P) % IW
                xbw = xb_win(iww)
                for ch in range(CH):
                    nc.tensor.matmul(
                        ps_v,
                        xbw[:, ch, off : off + P],
                        wvt[:, ch, :],
                        start=(ch == 0), stop=(ch == CH - 1),
                    )
                if cast_on_act:
                    nc.scalar.copy(vtt, ps_v)
                else:
                    nc.vector.tensor_copy(vtt, ps_v)
                vt[jt] = vtt

            nc.vector.tensor_copy(bqk_sb[0:C8, :], bq_sb)
            nc.vector.tensor_copy(bqk_sb[C8 : 2 * C8, :], bk_sb)
            emit_xcast(2)
            emit_qkproj(0)
            for jt in range(4):
                emit_vproj(jt)
            emit_xcast(1)
            emit_qkproj(1)
            # gbv = gamma * bv via on-chip transpose of the fast-shape load
            ps_bv = pe_ps.tile([P, CH], F32, tag="peps", name="ps_bv")
            nc.tensor.transpose(ps_bv, bv2_sb, ident[:CH, :CH])
            nc.vector.tensor_scalar_mul(gbv, ps_bv, gamma_sb)
            # rowsum stationary carries 1/gamma so rinv = gamma / rowsum
            giv = const.tile([P, 1], F32, tag="giv")
            nc.vector.reciprocal(giv, gamma_sb)
            ones_g = const.tile([P, P], BF16, tag="onesg")
            nc.vector.tensor_scalar_mul(ones_g, ones_bf, giv)
            for jt in range(4, 8):
                emit_vproj(jt)
            emit_xcast(3)
            vjt_late = list(range(8, JT))
            qk_late = list(range(2, NW))
            xc_late = list(range(4, NW))

            # ---------------- main pipeline ----------------
            state = {}
            pse_hist = {}

            def emit_group(g):
                iw, gg = divmod(g, GPW)
                win = bass.ts(iw, IW)
                if gg == 0:
                    state[iw] = {
                        "av": av_ps.tile(
                            [P, CH, IW], F32, tag="avps", name=f"av_{iw}"
                        ),
                        "acc": accpool.tile(
                            [P, 2, IW], BF16, tag="acc", name=f"acc_{iw}"
                        ),
                    }
                ps_e = pe_ps.tile([P, 2, IW], F32, tag="peps", name=f"ps_e{g}")
                pse_hist[g] = ps_e
                pse_hist.pop(g - 4, None)
                for m in range(2):
                    jt = 2 * gg + m
                    nc.tensor.matmul(
                        ps_e[:, m, :],
                        k4[m * C8 : (m + 1) * C8, bass.ts(jt, P)],
                        q4[m * C8 : (m + 1) * C8, win],
                        start=True, stop=True,
                        tile_position=(m * C8, 0),
                    )
                pt = ptpool.tile([P, 2, IW], BF16, tag="pt", name=f"pt{g}")
                nc.scalar.activation(pt, ps_e, mybir.ActivationFunctionType.Exp)
                acc = state[iw]["acc"]
                if gg == 0:
                    nc.vector.tensor_copy(acc, pt)
                else:
                    nc.vector.tensor_add(acc, acc, pt)
                return pt

            def emit_av(g, pt):
                iw, gg = divmod(g, GPW)
                av = state[iw]["av"]
                for m in range(2):
                    jt = 2 * gg + m
                    for ch in range(CH):
                        nc.tensor.matmul(
                            av[:, ch, :],
                            vt[jt][:, bass.ts(ch, P)],
                            pt[:, m, :],
                            start=(gg == 0 and m == 0),
                            stop=(gg == GPW - 1 and m == 1),
                            skip_group_check=True,
                        )

            def emit_epilogue(iw, cur_g):
                st = state.pop(iw)
                acc, av = st["acc"], st["av"]
                win = bass.ts(iw, IW)
                # rowsum goes into the previous group's ps_e tile: its exp
                # has already drained it, and the tile's next QK writer is
                # two groups out, past the reciprocal read below.  Reusing a
                # live tile (instead of allocating) keeps the pe_ps rotation
                # parity intact, so no group ever single-buffers.
                host = max(k for k in pse_hist if k <= cur_g - 1)
                ps_r = pse_hist[host][:, 0, :]
                for s in range(2):
                    nc.tensor.matmul(
                        ps_r, ones_g, acc[:, s, :],
                        start=(s == 0), stop=(s == 1),
                    )
                rinv = smallwork.tile([P, IW], F32, tag="rinv", name=f"rinv{iw}")
                nc.vector.reciprocal_approx_fast(rinv, ps_r)
                xw = x_win(iw)
                for ch in range(CH):
                    o_sb = outpool.tile([P, IW], F32, tag="osb", name=f"osb{ch}_{iw}")
                    nc.vector.tensor_mul(o_sb, av[:, ch, :], rinv)
                    nc.vector.scalar_tensor_tensor(
                        out=o_sb, in0=o_sb, scalar=gbv[:, ch : ch + 1],
                        in1=xw[:, ch, :],
                        op0=mybir.AluOpType.add, op1=mybir.AluOpType.add,
                    )
                    eng = nc.sync if ch == 0 else nc.gpsimd
                    eng.dma_start(
                        out=out_d[ch * P : (ch + 1) * P, win], in_=o_sb
                    )

            pts = [None] * NG
            for g in range(NG + 1):
                if g < NG:
                    if xc_late:
                        emit_xcast(xc_late.pop(0))
                    pts[g] = emit_group(g)
                if g > 0:
                    emit_av(g - 1, pts[g - 1])
                    pts[g - 1] = None
                if g < NG:
                    # drips go AFTER emit_av so their psum-rotation steals
                    # never stall the PE ahead of the AV matmuls
                    if qk_late:
                        emit_qkproj(qk_late.pop(0))
                    for _ in range(3):
                        if vjt_late:
                            emit_vproj(vjt_late.pop(0))
                if g > 0 and g - 1 >= GPW + 1 and (g - 1) % GPW == 1:
                    emit_epilogue((g - 1) // GPW - 1, g)
            emit_epilogue(NW - 1, NG)

    nc.finalize()
    return nc


_NC_CACHE: dict[int, bass.Bass] = {}


def _get_nc(n: int) -> bass.Bass:
    if n not in _NC_CACHE:
        _NC_CACHE[n] = build_attention_nc(n)
    return _NC_CACHE[n]


def kernel(x, Wq, bq, Wk, bk, Wv, bv, gamma):
    B, c, h, w = x.shape
    n = h * w
    assert B == 8 and c == C
    nc = _get_nc(n)
    xf = np.ascontiguousarray(np.asarray(x, dtype=np.float32).reshape(B, c, n))
    common = {
        "Wq": np.ascontiguousarray(np.asarray(Wq, dtype=np.float32)),
        "bq": np.ascontiguousarray(np.asarray(bq, dtype=np.float32)),
        "Wk": np.ascontiguousarray(np.asarray(Wk, dtype=np.float32)),
        "bk": np.ascontiguousarray(np.asarray(bk, dtype=np.float32)),
        "Wv": np.ascontiguousarray(np.asarray(Wv, dtype=np.float32)),
        "bv": np.ascontiguousarray(np.asarray(bv, dtype=np.float32)),
        "gamma": np.ascontiguousarray(np.asarray(gamma, dtype=np.float32)),
    }
    in_maps = [{"x": xf[b], **common} for b in range(B)]
    res = run_bass_kernel_spmd(nc, in_maps, core_ids=list(range(B)))
    out = np.stack([res.results[b]["out"].reshape(c, h, w) for b in range(B)])
    return out.astype(np.float32)
